# revision 37
# baseline (speedup 1.0000x reference)
"""AttentionalPropagation (SuperGlue-style GNN message passing) on 8 trn2 cores.

Problem (hardcoded): B=2, D=256, N=M=4096, H=4 heads, head dim 64.
  q = P_q(x); k = P_k(source); v = P_v(source)      (bottleneck 1x1 convs D->D/8->D)
  msg = attn(q, k, v); merged = P_m(msg)            (per-head softmax over M)
  out = Conv(relu(BN(Conv(cat[x, merged]))))        (512->64->256)

Sharding: 8 cores = (batch b in {0,1}) x (query chunk of 1024).  Each core
computes k/v for its full batch row (cheap, duplicated 4x) and attention +
MLP for its 1024 query columns.  Weights replicated.  No collectives.

Layout: channels-on-partitions everywhere except attention scores, which are
computed transposed (keys m on partitions, queries n free) so softmax
normalization folds into the PE: the value matrix vT carries an extra
all-ones column per head, making row 64 of the msg-PSUM the softmax
denominator.  Head channels are made contiguous by permuting weight rows/cols
on the host.

Dtypes: attention path runs bf16 (error is attenuated: msg is a small additive
contribution vs x); the x -> MLP -> out path runs float32r.

HAM note: trn2's PE clock-gate only counts *full-K* (128-partition) matmuls as
activity; K<=64 matmuls run at 1.2 GHz forever.  So every hot matmul here is
padded to K=128 with zeros placed in the host-prepared weights (zero rows
contract against garbage-free operands), and the per-head scores matmul
contracts both heads' k against a zero-masked q.
"""

import numpy as np

import concourse.bass as bass
import concourse.mybir as mybir
import concourse.tile as tile
from concourse import bacc, bass_utils

B, D, N, M, H = 2, 256, 4096, 4096, 4
DIM = D // H       # 64
D8 = D // 8        # 32
TD = 2 * D         # 512
TD8 = TD // 8      # 64
BN_EPS = 1e-5
NCORES = 8
NCHUNK = N // 4    # query columns per core
NT = 512           # n tile (PSUM bank = 512 fp32)
NTILES = NCHUNK // NT          # 2
MT = 512           # source m tile for k/v projection stage
MTILES = M // MT               # 8
MC = 128           # m chunk (scores PSUM partition dim)
MCHUNKS = M // MC              # 32
BC = 2             # score chunks per exp batch (amortize ACT fixed cost)
NBATCH = MCHUNKS // BC
F32 = mybir.dt.float32
F32R = mybir.dt.float32r
BF16 = mybir.dt.bfloat16
AF = mybir.ActivationFunctionType


def _mm(nc, out, lhsT, rhs, start, stop):
    nc.tensor.matmul(out, lhsT, rhs, start=start, stop=stop)


def build_body(ctx, tc: tile.TileContext, io):
    nc = tc.nc
    x_d = io["x_chunk"]          # [2, 128, NCHUNK]  (channel-chunk, partition, n)
    src_d = io["source_b"]       # [2, 128, M]
    out_d = io["out_chunk"]      # [2, 128, NCHUNK]

    consts = ctx.enter_context(tc.tile_pool(name="consts", bufs=1))
    big = ctx.enter_context(tc.tile_pool(name="big", bufs=1))
    srcp = ctx.enter_context(tc.tile_pool(name="srcp", bufs=3))
    kv1p = ctx.enter_context(tc.tile_pool(name="kv1p", bufs=1))
    ep = ctx.enter_context(tc.tile_pool(name="ep", bufs=6))
    nrm = ctx.enter_context(tc.tile_pool(name="nrm", bufs=4))
    pp = ctx.enter_context(tc.tile_pool(name="pp", bufs=2, space="PSUM"))

    # ---- weights (host-preprocessed; zero-padded to K=128 where noted) ----
    def wtile(name, shape, dt=F32R):
        t = consts.tile(shape, dt, name=name)
        nc.gpsimd.dma_start(out=t, in_=io[name])
        return t

    # stage-B weights first: their DMAs gate the first real matmuls
    ones_row = wtile("ones", [1, NCHUNK])
    wk1t = wtile("wk1t", [128, 2, D8], BF16)
    wv1t = wtile("wv1t", [128, 2, D8], BF16)
    bk1 = wtile("bk1", [D8, 1], F32)
    bv1 = wtile("bv1", [D8, 1], F32)
    rvp = wtile("rvp", [128, H * 128], BF16)      # rows 64-95 Wv2'T, 96 bias/ones
    x_sb = big.tile([128, 2, NCHUNK], F32R)
    for _ct in range(2):
        nc.gpsimd.dma_start(out=x_sb[:, _ct, :], in_=x_d[_ct])
    wq1t = wtile("wq1t", [128, 2, D8])            # f32r (x path)
    bq1 = wtile("bq1", [D8, 1], F32)
    cht = wtile("cht", [128, H, 128], BF16)       # C_h^T: scores = k1'^T C_h q1'
    wm1t = wtile("wm1t", [128, H, D8], BF16)      # rows 64-127 zero
    bm1 = wtile("bm1", [1, D8])
    wm2t = wtile("wm2t", [128, 2, 128], BF16)
    wp1xt = wtile("wp1xt", [128, 2, TD8])         # f32r
    wp1mt = wtile("wp1mt", [128, 2, TD8], BF16)
    bp1 = wtile("bp1", [1, TD8])
    g1s = wtile("g1s", [TD8, 1], F32)
    be1 = wtile("be1", [TD8, 1], F32)
    wp2t = wtile("wp2t", [TD8 + 1, 2, 128])       # f32r

    # ---- PE warm-up: HAM un-throttles only under sustained full-K matmul
    # activity; run zero matmuls while the input DMAs stream in.
    wza = consts.tile([128, 128], BF16)
    wzb = consts.tile([128, NT], BF16)
    nc.vector.memset(wza, 0.0)
    nc.vector.memset(wzb, 0.0)
    for i in range(26):
        pw = pp.tile([128, NT], F32, tag="pp", name="pw")
        _mm(nc, pw, wza, wzb, True, True)

    # ---- persistent activations ----
    kv1_all = big.tile([128, M], BF16)   # rows: 0-31 k1, 32 ones, 64-95 v1,
    nc.vector.memset(kv1_all[32:64, :], 0.0)       # 96 ones, rest zero
    nc.vector.memset(kv1_all[96:128, :], 0.0)
    nc.vector.memset(kv1_all[D8:D8 + 1, :], 1.0)
    nc.vector.memset(kv1_all[96:97, :], 1.0)
    vT_sb = big.tile([128, MCHUNKS, H, 128], BF16)     # [v'|ones|0pad] per head
    qh_sb = big.tile([128, H, NCHUNK], BF16)           # C_h^T q1', rows 33+ zero
    msg_sb = big.tile([128, H, NCHUNK], BF16)

    # ---- k / v projections (full M, streamed in m tiles, SW-pipelined) ----
    # kv1_all doubles as the scores lhsT (k2 is folded into host-side C_h).
    # vT consumption of tile mt is emitted after mt+1's k1/v1 matmuls so the
    # PE never waits on the DVE bias-add.
    ppb = tc.tile_pool(name="ppb", bufs=3, space="PSUM")
    ppb_pool = ppb.__enter__()

    def emit_kv1(mt):
        ms = mt * MT
        src = srcp.tile([128, 2, MT], BF16, tag="src", name="src")
        for ct in range(2):
            nc.sync.dma_start(out=src[:, ct, :], in_=src_d[ct, :, ms:ms + MT])
        for (w1, b1, r0) in ((wk1t, bk1, 0), (wv1t, bv1, 64)):
            ps1 = ppb_pool.tile([D8, MT], F32, tag="ps1", name="ps1")
            _mm(nc, ps1, w1[:, 0, :], src[:, 0, :], True, False)
            _mm(nc, ps1, w1[:, 1, :], src[:, 1, :], False, True)
            nc.vector.tensor_scalar_add(out=kv1_all[r0:r0 + D8, ms:ms + MT],
                                        in0=ps1, scalar1=b1)

    def emit_kv2(mt):
        ms = mt * MT
        for j in range(MT // MC):
            mc = (ms // MC) + j
            psv = ppb_pool.tile([128, H, 128], F32, tag="pskv", name="psv")
            _mm(nc, psv, kv1_all[:, mc * MC:(mc + 1) * MC], rvp, True, True)
            nc.scalar.copy(out=vT_sb[:, mc, :, :], in_=psv)

    prev = None
    for mt in range(MTILES):
        emit_kv1(mt)
        if prev is not None:
            emit_kv2(prev)
        prev = mt
    emit_kv2(prev)

    # ---- q projection (this core's n chunk) ----
    nc.gpsimd.memset(msg_sb[64:128, :, :], 0.0)  # rows 65+ zero; 64 overwritten
    q1 = big.tile([128, NCHUNK], BF16)        # rows 0-31 q1, 32 ones, rest 0
    nc.gpsimd.memset(q1[32:64, :], 0.0)
    nc.gpsimd.memset(q1[64:128, :], 0.0)
    for nt in range(NTILES):
        ns = nt * NT
        psq = pp.tile([D8, NT], F32, tag="pp", name="psq")
        _mm(nc, psq, wq1t[:, 0, :], x_sb[:, 0, ns:ns + NT], True, False)
        _mm(nc, psq, wq1t[:, 1, :], x_sb[:, 1, ns:ns + NT], False, True)
        nc.vector.tensor_scalar_add(out=q1[0:D8, ns:ns + NT], in0=psq, scalar1=bq1)
    nc.vector.tensor_copy(out=q1[D8:D8 + 1, :], in_=ones_row)
    for h in range(H):
        for nt in range(NTILES):
            ns = nt * NT
            psq2 = pp.tile([128, NT], F32, tag="pp", name="psq2")
            _mm(nc, psq2, cht[:, h, :], q1[:, ns:ns + NT], True, True)
            nc.vector.tensor_copy(out=qh_sb[:, h, ns:ns + NT], in_=psq2)

    # stage-B psum pool released; attention pools take its banks
    ppb.__exit__(None, None, None)
    pps = ctx.enter_context(tc.tile_pool(name="pps", bufs=2, space="PSUM"))
    ppm = ctx.enter_context(tc.tile_pool(name="ppm", bufs=2, space="PSUM"))

    # ---- attention + merge + MLP (flat pipeline over (nt, h, batch)) ----
    # scores^T chunk [m=128, n=NT]: full-K matmul of both heads' k against the
    # zero-masked q of head h.  exp on ACT (scale folds 1/sqrt(DIM)), BC chunks
    # per instruction.  msg psum accumulates vT' @ exp; row 64 = denominator.
    # The scores/exp for pipeline step i+1 are emitted before the msg matmuls
    # of step i -- across head boundaries too -- so the PE queue stays dense.
    m1 = big.tile([128, NCHUNK], BF16)        # rows 0-31 + ones row 32, rest 0
    nc.gpsimd.memset(m1[32:64, :], 0.0)
    nc.gpsimd.memset(m1[64:128, :], 0.0)
    nc.vector.tensor_copy(out=m1[D8:D8 + 1, :], in_=ones_row)
    mm_sb = big.tile([128, 2, NCHUNK], BF16)      # merged msg, unpermuted chans
    h1 = big.tile([TD8 + 1, NCHUNK], F32R)
    nc.vector.tensor_copy(out=h1[TD8:TD8 + 1, :], in_=ones_row)
    out_sb = big.tile([128, 2, NCHUNK], F32)

    def emit_scores(nt, h, bi):
        ns = nt * NT
        ps = pps.tile([128, BC, NT], F32, tag="ps", name="ps")
        for j in range(BC):
            mc = bi * BC + j
            _mm(nc, ps[:, j, :], kv1_all[:, mc * MC:(mc + 1) * MC],
                qh_sb[:, h, ns:ns + NT], True, True)
        e = ep.tile([128, BC, NT], BF16, tag="e", name="e")
        nc.scalar.activation(out=e, in_=ps, func=AF.Exp, scale=0.125)
        return e

    def emit_norm(pm, h, ns):
        rec = nrm.tile([1, NT], F32, tag="rec", name="rec")
        nc.vector.reciprocal_approx_fast(out=rec, in_=pm[0:1, :])
        bc = nrm.tile([DIM + 1, NT], F32, tag="bc", name="bc")
        nc.gpsimd.partition_broadcast(bc, rec)
        nc.vector.tensor_mul(out=msg_sb[0:DIM + 1, h, ns:ns + NT],
                             in0=pm[0:DIM + 1, :], in1=bc)

    def emit_merge_mlp(nt):
        ns = nt * NT
        psm = pp.tile([D8, NT], F32, tag="pp", name="psm")
        for h in range(H):
            _mm(nc, psm, wm1t[:, h, :], msg_sb[:, h, ns:ns + NT], h == 0, False)
        _mm(nc, psm, bm1, ones_row[:, 0:NT], False, True)
        nc.vector.tensor_copy(out=m1[0:D8, ns:ns + NT], in_=psm)
        for ct in range(2):
            psm2 = pp.tile([128, NT], F32, tag="pp", name="psm2")
            _mm(nc, psm2, wm2t[:, ct, :], m1[:, ns:ns + NT], True, True)
            nc.vector.tensor_copy(out=mm_sb[:, ct, ns:ns + NT], in_=psm2)
        psh = pp.tile([TD8, NT], F32, tag="pp", name="psh")
        _mm(nc, psh, wp1xt[:, 0, :], x_sb[:, 0, ns:ns + NT], True, False)
        _mm(nc, psh, wp1xt[:, 1, :], x_sb[:, 1, ns:ns + NT], False, False)
        _mm(nc, psh, wp1mt[:, 0, :], mm_sb[:, 0, ns:ns + NT], False, False)
        _mm(nc, psh, wp1mt[:, 1, :], mm_sb[:, 1, ns:ns + NT], False, False)
        _mm(nc, psh, bp1, ones_row[:, 0:NT], False, True)
        nc.scalar.activation(out=h1[0:TD8, ns:ns + NT], in_=psh, func=AF.Relu,
                             bias=be1, scale=g1s)
        for ct in range(2):
            pso = pp.tile([128, NT], F32, tag="pp", name="pso")
            _mm(nc, pso, wp2t[:, ct, :], h1[:, ns:ns + NT], True, True)
            nc.vector.tensor_copy(out=out_sb[:, ct, ns:ns + NT], in_=pso)
            nc.sync.dma_start(out=out_d[ct, :, ns:ns + NT],
                              in_=out_sb[:, ct, ns:ns + NT])

    seq = [(nt, h, bi) for nt in range(NTILES) for h in range(H)
           for bi in range(NBATCH)]
    pend = emit_scores(*seq[0])
    pm = None
    for idx, (nt, h, bi) in enumerate(seq):
        nxt = emit_scores(*seq[idx + 1]) if idx + 1 < len(seq) else None
        if bi == 0:
            pm = ppm.tile([128, NT], F32, tag="pm", name="pm")
        for j in range(BC):
            mc = bi * BC + j
            _mm(nc, pm, vT_sb[:, mc, h, :],
                pend[:, j, :], mc == 0, mc == MCHUNKS - 1)
        if bi == NBATCH - 1:
            emit_norm(pm, h, nt * NT)
            if h == H - 1:
                emit_merge_mlp(nt)
        pend = nxt


def build_program():
    nc = bacc.Bacc("TRN2", target_bir_lowering=False, debug=False)
    io = {}
    def inp(name, shape, dt=F32R):
        io[name] = nc.dram_tensor(name, shape, dt, kind="ExternalInput").ap()
    inp("x_chunk", [2, 128, NCHUNK])
    inp("source_b", [2, 128, M], BF16)
    inp("wq1t", [128, 2, D8]); inp("bq1", [D8, 1], F32)
    inp("wk1t", [128, 2, D8], BF16); inp("bk1", [D8, 1], F32)
    inp("wv1t", [128, 2, D8], BF16); inp("bv1", [D8, 1], F32)
    inp("cht", [128, H, 128], BF16)
    inp("rvp", [128, H * 128], BF16)
    inp("wm1t", [128, H, D8], BF16); inp("bm1", [1, D8])
    inp("wm2t", [128, 2, 128], BF16)
    inp("wp1xt", [128, 2, TD8]); inp("wp1mt", [128, 2, TD8], BF16)
    inp("bp1", [1, TD8])
    inp("g1s", [TD8, 1], F32); inp("be1", [TD8, 1], F32)
    inp("wp2t", [TD8 + 1, 2, 128])
    inp("ones", [1, NCHUNK])
    io["out_chunk"] = nc.dram_tensor(
        "out_chunk", [2, 128, NCHUNK], F32, kind="ExternalOutput").ap()
    from contextlib import ExitStack
    with tile.TileContext(nc) as tc, ExitStack() as ctx:
        build_body(ctx, tc, io)
    nc.compile()
    return nc


def prep_weights(i):
    """Host-side preprocessing: transposes, head-channel permutation, bias
    folding (extra contraction rows), K=128 zero padding, BN folding."""
    import ml_dtypes
    bf = ml_dtypes.bfloat16
    f = np.float32
    a = {k: np.asarray(v, dtype=f) for k, v in i.items()}
    # permutation making head channels contiguous: c' = h*64+d  <- c = 4*d+h
    perm = (np.arange(H)[:, None] + H * np.arange(DIM)[None, :]).reshape(-1)

    def w1t(w):       # [D8, D] -> [128, 2, D8]
        return np.ascontiguousarray(w.T.reshape(2, 128, D8).swapaxes(0, 1))

    def w2tp(w, b):   # [D, D8] x [D] -> [128, 2, 128]: rows [w.T; b; zeros]
        o = np.zeros((128, 2, 128), f)
        o[0:D8] = w.T.reshape(D8, 2, 128)
        o[D8] = b.reshape(2, 128)
        return o

    out = {
        "wq1t": w1t(a["Wq1"]), "bq1": a["bq1"].reshape(D8, 1),
        "wk1t": w1t(a["Wk1"]), "bk1": a["bk1"].reshape(D8, 1),
        "wv1t": w1t(a["Wv1"]), "bv1": a["bv1"].reshape(D8, 1),
        "wm2t": w2tp(a["Wm2"], a["bm2"]),
        "wp2t": np.ascontiguousarray(np.concatenate(
            [a["Wp2"].T.reshape(TD8, 2, 128), a["bp2"].reshape(1, 2, 128)], 0)),
        "bm1": a["bm1"].reshape(1, D8),
        "bp1": a["bp1"].reshape(1, TD8),
        "g1s": (a["g1"] / np.sqrt(f(1.0) + f(BN_EPS))).reshape(TD8, 1).astype(f),
        "be1": a["be1"].reshape(TD8, 1),
        "ones": np.ones((1, NCHUNK), f),
    }
    # rvp [128, H*128]: kv1 layout has v1 at rows 64-95, ones at row 96.
    # per head h: cols [128h, 128h+64) = v weights; col 128h+64 = ones col
    # (softmax denominator); cols 128h+65.. zero.
    wv2p, bv2p = a["Wv2"][perm], a["bv2"][perm]
    rvp = np.zeros((128, H * 128), f)
    for h in range(H):
        c0 = h * 128
        rvp[64:64 + D8, c0 + 1:c0 + 1 + DIM] = wv2p[h * DIM:(h + 1) * DIM].T
        rvp[96, c0 + 1:c0 + 1 + DIM] = bv2p[h * DIM:(h + 1) * DIM]
        rvp[96, c0] = 1.0
    out["rvp"] = rvp
    # cht[:, h, :] = C_h^T (zero-padded to 128x128), C_h = A_h @ B_h.T where
    # A/B are the bias-extended per-head blocks of Wk2'/Wq2' transposed.
    wq2e = np.concatenate([a["Wq2"][perm].T, a["bq2"][perm][None, :]], 0)  # [33, 256]
    wk2e = np.concatenate([a["Wk2"][perm].T, a["bk2"][perm][None, :]], 0)
    cht = np.zeros((128, H, 128), f)
    for h in range(H):
        A = wk2e[:, h * DIM:(h + 1) * DIM]        # [33, 64]
        Bq = wq2e[:, h * DIM:(h + 1) * DIM]
        C = (A.astype(np.float64) @ Bq.astype(np.float64).T).astype(f)  # [33,33]
        cht[0:D8 + 1, h, 0:D8 + 1] = C.T
    out["cht"] = cht
    # wm1t [128, 4, D8]: [d, h, :] = Wm1'[:, h*64+d] for d<64, zeros below
    wm1p = a["Wm1"][:, perm]
    wm1t = np.zeros((128, H, D8), f)
    wm1t[1:DIM + 1] = wm1p.T.reshape(H, DIM, D8).swapaxes(0, 1)
    out["wm1t"] = wm1t
    # mlp conv1 split into x-part and msg-part
    out["wp1xt"] = np.ascontiguousarray(
        a["Wp1"][:, 0:D].T.reshape(2, 128, TD8).swapaxes(0, 1))
    out["wp1mt"] = np.ascontiguousarray(
        a["Wp1"][:, D:TD].T.reshape(2, 128, TD8).swapaxes(0, 1))
    bf16_names = {"wk1t", "wv1t", "cht", "rvp", "wm1t", "wm2t", "wp1mt"}
    return {k: np.ascontiguousarray(v.astype(bf) if k in bf16_names else v)
            for k, v in out.items()}


_NC_CACHE = None


def _get_nc():
    global _NC_CACHE
    if _NC_CACHE is None:
        _NC_CACHE = build_program()
    return _NC_CACHE


def make_in_maps(inputs):
    import ml_dtypes
    w = prep_weights(inputs)
    x = np.ascontiguousarray(np.asarray(inputs["x"], np.float32))
    src = np.ascontiguousarray(np.asarray(inputs["source"], np.float32))
    in_maps = []
    for c in range(NCORES):
        b, ns = c // 4, (c % 4) * NCHUNK
        m = dict(w)
        m["x_chunk"] = np.ascontiguousarray(
            x[b].reshape(2, 128, N)[:, :, ns:ns + NCHUNK])
        m["source_b"] = np.ascontiguousarray(src[b].reshape(2, 128, M)).astype(
            ml_dtypes.bfloat16)
        in_maps.append(m)
    return in_maps


def assemble_out(results):
    out = np.empty((B, D, N), np.float32)
    for c in range(NCORES):
        b, ns = c // 4, (c % 4) * NCHUNK
        out[b].reshape(2, 128, N)[:, :, ns:ns + NCHUNK] = (
            results[c]["out_chunk"])
    return out


def kernel(**inputs):
    nc = _get_nc()
    res = bass_utils.run_bass_kernel_spmd(
        nc, make_in_maps(inputs), core_ids=list(range(NCORES)))
    return assemble_out(res.results)


# revision 38
# speedup vs baseline: 1.0023x; 1.0023x over previous
"""AttentionalPropagation (SuperGlue-style GNN message passing) on 8 trn2 cores.

Problem (hardcoded): B=2, D=256, N=M=4096, H=4 heads, head dim 64.
  q = P_q(x); k = P_k(source); v = P_v(source)      (bottleneck 1x1 convs D->D/8->D)
  msg = attn(q, k, v); merged = P_m(msg)            (per-head softmax over M)
  out = Conv(relu(BN(Conv(cat[x, merged]))))        (512->64->256)

Sharding: 8 cores = (batch b in {0,1}) x (query chunk of 1024).  Each core
computes k/v for its full batch row (cheap, duplicated 4x) and attention +
MLP for its 1024 query columns.  Weights replicated.  No collectives.

Layout: channels-on-partitions everywhere except attention scores, which are
computed transposed (keys m on partitions, queries n free) so softmax
normalization folds into the PE: the value matrix vT carries an extra
all-ones column per head, making row 64 of the msg-PSUM the softmax
denominator.  Head channels are made contiguous by permuting weight rows/cols
on the host.

Dtypes: attention path runs bf16 (error is attenuated: msg is a small additive
contribution vs x); the x -> MLP -> out path runs float32r.

HAM note: trn2's PE clock-gate only counts *full-K* (128-partition) matmuls as
activity; K<=64 matmuls run at 1.2 GHz forever.  So every hot matmul here is
padded to K=128 with zeros placed in the host-prepared weights (zero rows
contract against garbage-free operands), and the per-head scores matmul
contracts both heads' k against a zero-masked q.
"""

import numpy as np

import concourse.bass as bass
import concourse.mybir as mybir
import concourse.tile as tile
from concourse import bacc, bass_utils

B, D, N, M, H = 2, 256, 4096, 4096, 4
DIM = D // H       # 64
D8 = D // 8        # 32
TD = 2 * D         # 512
TD8 = TD // 8      # 64
BN_EPS = 1e-5
NCORES = 8
NCHUNK = N // 4    # query columns per core
NT = 512           # n tile (PSUM bank = 512 fp32)
NTILES = NCHUNK // NT          # 2
MT = 512           # source m tile for k/v projection stage
MTILES = M // MT               # 8
MC = 128           # m chunk (scores PSUM partition dim)
MCHUNKS = M // MC              # 32
BC = 2             # score chunks per exp batch (amortize ACT fixed cost)
NBATCH = MCHUNKS // BC
F32 = mybir.dt.float32
F32R = mybir.dt.float32r
BF16 = mybir.dt.bfloat16
AF = mybir.ActivationFunctionType


def _mm(nc, out, lhsT, rhs, start, stop):
    nc.tensor.matmul(out, lhsT, rhs, start=start, stop=stop)


def build_body(ctx, tc: tile.TileContext, io):
    nc = tc.nc
    x_d = io["x_chunk"]          # [2, 128, NCHUNK]  (channel-chunk, partition, n)
    src_d = io["source_b"]       # [2, 128, M]
    out_d = io["out_chunk"]      # [2, 128, NCHUNK]

    consts = ctx.enter_context(tc.tile_pool(name="consts", bufs=1))
    big = ctx.enter_context(tc.tile_pool(name="big", bufs=1))
    srcp = ctx.enter_context(tc.tile_pool(name="srcp", bufs=3))
    kv1p = ctx.enter_context(tc.tile_pool(name="kv1p", bufs=1))
    ep = ctx.enter_context(tc.tile_pool(name="ep", bufs=6))
    nrm = ctx.enter_context(tc.tile_pool(name="nrm", bufs=4))
    pp = ctx.enter_context(tc.tile_pool(name="pp", bufs=2, space="PSUM"))

    # ---- weights (host-preprocessed; zero-padded to K=128 where noted) ----
    def wtile(name, shape, dt=F32R):
        t = consts.tile(shape, dt, name=name)
        nc.gpsimd.dma_start(out=t, in_=io[name])
        return t

    # stage-B weights first: their DMAs gate the first real matmuls
    ones_row = wtile("ones", [1, NCHUNK])
    wk1t = wtile("wk1t", [128, 2, D8], BF16)
    wv1t = wtile("wv1t", [128, 2, D8], BF16)
    bk1 = wtile("bk1", [D8, 1], F32)
    bv1 = wtile("bv1", [D8, 1], F32)
    rvp = wtile("rvp", [128, H * 128], BF16)      # rows 64-95 Wv2'T, 96 bias/ones
    wq1t = wtile("wq1t", [128, 2, D8])            # f32r (x path)
    bq1 = wtile("bq1", [D8, 1], F32)
    cht = wtile("cht", [128, H, 128], BF16)       # C_h^T: scores = k1'^T C_h q1'
    wm1t = wtile("wm1t", [128, H, D8], BF16)      # rows 64-127 zero
    bm1 = wtile("bm1", [1, D8])
    wm2t = wtile("wm2t", [128, 2, 128], BF16)
    wp1xt = wtile("wp1xt", [128, 2, TD8])         # f32r
    wp1mt = wtile("wp1mt", [128, 2, TD8], BF16)
    bp1 = wtile("bp1", [1, TD8])
    g1s = wtile("g1s", [TD8, 1], F32)
    be1 = wtile("be1", [TD8, 1], F32)
    wp2t = wtile("wp2t", [TD8 + 1, 2, 128])       # f32r

    # ---- PE warm-up: HAM un-throttles only under sustained full-K matmul
    # activity; run zero matmuls while the input DMAs stream in.
    wza = consts.tile([128, 128], BF16)
    wzb = consts.tile([128, NT], BF16)
    nc.vector.memset(wza, 0.0)
    nc.vector.memset(wzb, 0.0)
    for i in range(26):
        pw = pp.tile([128, NT], F32, tag="pp", name="pw")
        _mm(nc, pw, wza, wzb, True, True)

    # ---- persistent activations ----
    x_sb = big.tile([128, 2, NCHUNK], F32R)
    for _ct in range(2):
        nc.sync.dma_start(out=x_sb[:, _ct, :], in_=x_d[_ct])
    kv1_all = big.tile([128, M], BF16)   # rows: 0-31 k1, 32 ones, 64-95 v1,
    nc.vector.memset(kv1_all[32:64, :], 0.0)       # 96 ones, rest zero
    nc.vector.memset(kv1_all[96:128, :], 0.0)
    nc.vector.memset(kv1_all[D8:D8 + 1, :], 1.0)
    nc.vector.memset(kv1_all[96:97, :], 1.0)
    vT_sb = big.tile([128, MCHUNKS, H, 128], BF16)     # [v'|ones|0pad] per head
    qh_sb = big.tile([128, H, NCHUNK], BF16)           # C_h^T q1', rows 33+ zero
    msg_sb = big.tile([128, H, NCHUNK], BF16)

    # ---- k / v projections (full M, streamed in m tiles, SW-pipelined) ----
    # kv1_all doubles as the scores lhsT (k2 is folded into host-side C_h).
    # vT consumption of tile mt is emitted after mt+1's k1/v1 matmuls so the
    # PE never waits on the DVE bias-add.
    ppb = tc.tile_pool(name="ppb", bufs=3, space="PSUM")
    ppb_pool = ppb.__enter__()

    def emit_kv1(mt):
        ms = mt * MT
        src = srcp.tile([128, 2, MT], BF16, tag="src", name="src")
        for ct in range(2):
            nc.sync.dma_start(out=src[:, ct, :], in_=src_d[ct, :, ms:ms + MT])
        for (w1, b1, r0) in ((wk1t, bk1, 0), (wv1t, bv1, 64)):
            ps1 = ppb_pool.tile([D8, MT], F32, tag="ps1", name="ps1")
            _mm(nc, ps1, w1[:, 0, :], src[:, 0, :], True, False)
            _mm(nc, ps1, w1[:, 1, :], src[:, 1, :], False, True)
            nc.vector.tensor_scalar_add(out=kv1_all[r0:r0 + D8, ms:ms + MT],
                                        in0=ps1, scalar1=b1)

    def emit_kv2(mt):
        ms = mt * MT
        for j in range(MT // MC):
            mc = (ms // MC) + j
            psv = ppb_pool.tile([128, H, 128], F32, tag="pskv", name="psv")
            _mm(nc, psv, kv1_all[:, mc * MC:(mc + 1) * MC], rvp, True, True)
            nc.scalar.copy(out=vT_sb[:, mc, :, :], in_=psv)

    prev = None
    for mt in range(MTILES):
        emit_kv1(mt)
        if prev is not None:
            emit_kv2(prev)
        prev = mt
    emit_kv2(prev)

    # ---- q projection (this core's n chunk) ----
    nc.gpsimd.memset(msg_sb[64:128, :, :], 0.0)  # rows 65+ zero; 64 overwritten
    q1 = big.tile([128, NCHUNK], BF16)        # rows 0-31 q1, 32 ones, rest 0
    nc.gpsimd.memset(q1[32:64, :], 0.0)
    nc.gpsimd.memset(q1[64:128, :], 0.0)
    for nt in range(NTILES):
        ns = nt * NT
        psq = pp.tile([D8, NT], F32, tag="pp", name="psq")
        _mm(nc, psq, wq1t[:, 0, :], x_sb[:, 0, ns:ns + NT], True, False)
        _mm(nc, psq, wq1t[:, 1, :], x_sb[:, 1, ns:ns + NT], False, True)
        nc.vector.tensor_scalar_add(out=q1[0:D8, ns:ns + NT], in0=psq, scalar1=bq1)
    nc.vector.tensor_copy(out=q1[D8:D8 + 1, :], in_=ones_row)
    for h in range(H):
        for nt in range(NTILES):
            ns = nt * NT
            psq2 = pp.tile([128, NT], F32, tag="pp", name="psq2")
            _mm(nc, psq2, cht[:, h, :], q1[:, ns:ns + NT], True, True)
            nc.vector.tensor_copy(out=qh_sb[:, h, ns:ns + NT], in_=psq2)

    # stage-B psum pool released; attention pools take its banks
    ppb.__exit__(None, None, None)
    pps = ctx.enter_context(tc.tile_pool(name="pps", bufs=2, space="PSUM"))
    ppm = ctx.enter_context(tc.tile_pool(name="ppm", bufs=2, space="PSUM"))

    # ---- attention + merge + MLP (flat pipeline over (nt, h, batch)) ----
    # scores^T chunk [m=128, n=NT]: full-K matmul of both heads' k against the
    # zero-masked q of head h.  exp on ACT (scale folds 1/sqrt(DIM)), BC chunks
    # per instruction.  msg psum accumulates vT' @ exp; row 64 = denominator.
    # The scores/exp for pipeline step i+1 are emitted before the msg matmuls
    # of step i -- across head boundaries too -- so the PE queue stays dense.
    m1 = big.tile([128, NCHUNK], BF16)        # rows 0-31 + ones row 32, rest 0
    nc.gpsimd.memset(m1[32:64, :], 0.0)
    nc.gpsimd.memset(m1[64:128, :], 0.0)
    nc.vector.tensor_copy(out=m1[D8:D8 + 1, :], in_=ones_row)
    mm_sb = big.tile([128, 2, NCHUNK], BF16)      # merged msg, unpermuted chans
    h1 = big.tile([TD8 + 1, NCHUNK], F32R)
    nc.vector.tensor_copy(out=h1[TD8:TD8 + 1, :], in_=ones_row)
    out_sb = big.tile([128, 2, NCHUNK], F32)

    def emit_scores(nt, h, bi):
        ns = nt * NT
        ps = pps.tile([128, BC, NT], F32, tag="ps", name="ps")
        for j in range(BC):
            mc = bi * BC + j
            _mm(nc, ps[:, j, :], kv1_all[:, mc * MC:(mc + 1) * MC],
                qh_sb[:, h, ns:ns + NT], True, True)
        e = ep.tile([128, BC, NT], BF16, tag="e", name="e")
        nc.scalar.activation(out=e, in_=ps, func=AF.Exp, scale=0.125)
        return e

    def emit_norm(pm, h, ns):
        rec = nrm.tile([1, NT], F32, tag="rec", name="rec")
        nc.vector.reciprocal_approx_fast(out=rec, in_=pm[0:1, :])
        bc = nrm.tile([DIM + 1, NT], F32, tag="bc", name="bc")
        nc.gpsimd.partition_broadcast(bc, rec)
        nc.vector.tensor_mul(out=msg_sb[0:DIM + 1, h, ns:ns + NT],
                             in0=pm[0:DIM + 1, :], in1=bc)

    def emit_merge_mlp(nt):
        ns = nt * NT
        psm = pp.tile([D8, NT], F32, tag="pp", name="psm")
        for h in range(H):
            _mm(nc, psm, wm1t[:, h, :], msg_sb[:, h, ns:ns + NT], h == 0, False)
        _mm(nc, psm, bm1, ones_row[:, 0:NT], False, True)
        nc.vector.tensor_copy(out=m1[0:D8, ns:ns + NT], in_=psm)
        for ct in range(2):
            psm2 = pp.tile([128, NT], F32, tag="pp", name="psm2")
            _mm(nc, psm2, wm2t[:, ct, :], m1[:, ns:ns + NT], True, True)
            nc.vector.tensor_copy(out=mm_sb[:, ct, ns:ns + NT], in_=psm2)
        psh = pp.tile([TD8, NT], F32, tag="pp", name="psh")
        _mm(nc, psh, wp1xt[:, 0, :], x_sb[:, 0, ns:ns + NT], True, False)
        _mm(nc, psh, wp1xt[:, 1, :], x_sb[:, 1, ns:ns + NT], False, False)
        _mm(nc, psh, wp1mt[:, 0, :], mm_sb[:, 0, ns:ns + NT], False, False)
        _mm(nc, psh, wp1mt[:, 1, :], mm_sb[:, 1, ns:ns + NT], False, False)
        _mm(nc, psh, bp1, ones_row[:, 0:NT], False, True)
        nc.scalar.activation(out=h1[0:TD8, ns:ns + NT], in_=psh, func=AF.Relu,
                             bias=be1, scale=g1s)
        for ct in range(2):
            pso = pp.tile([128, NT], F32, tag="pp", name="pso")
            _mm(nc, pso, wp2t[:, ct, :], h1[:, ns:ns + NT], True, True)
            nc.vector.tensor_copy(out=out_sb[:, ct, ns:ns + NT], in_=pso)
            nc.sync.dma_start(out=out_d[ct, :, ns:ns + NT],
                              in_=out_sb[:, ct, ns:ns + NT])

    seq = [(nt, h, bi) for nt in range(NTILES) for h in range(H)
           for bi in range(NBATCH)]
    pend = emit_scores(*seq[0])
    pm = None
    for idx, (nt, h, bi) in enumerate(seq):
        nxt = emit_scores(*seq[idx + 1]) if idx + 1 < len(seq) else None
        if bi == 0:
            pm = ppm.tile([128, NT], F32, tag="pm", name="pm")
        for j in range(BC):
            mc = bi * BC + j
            _mm(nc, pm, vT_sb[:, mc, h, :],
                pend[:, j, :], mc == 0, mc == MCHUNKS - 1)
        if bi == NBATCH - 1:
            emit_norm(pm, h, nt * NT)
            if h == H - 1:
                emit_merge_mlp(nt)
        pend = nxt


def build_program():
    nc = bacc.Bacc("TRN2", target_bir_lowering=False, debug=False)
    io = {}
    def inp(name, shape, dt=F32R):
        io[name] = nc.dram_tensor(name, shape, dt, kind="ExternalInput").ap()
    inp("x_chunk", [2, 128, NCHUNK])
    inp("source_b", [2, 128, M], BF16)
    inp("wq1t", [128, 2, D8]); inp("bq1", [D8, 1], F32)
    inp("wk1t", [128, 2, D8], BF16); inp("bk1", [D8, 1], F32)
    inp("wv1t", [128, 2, D8], BF16); inp("bv1", [D8, 1], F32)
    inp("cht", [128, H, 128], BF16)
    inp("rvp", [128, H * 128], BF16)
    inp("wm1t", [128, H, D8], BF16); inp("bm1", [1, D8])
    inp("wm2t", [128, 2, 128], BF16)
    inp("wp1xt", [128, 2, TD8]); inp("wp1mt", [128, 2, TD8], BF16)
    inp("bp1", [1, TD8])
    inp("g1s", [TD8, 1], F32); inp("be1", [TD8, 1], F32)
    inp("wp2t", [TD8 + 1, 2, 128])
    inp("ones", [1, NCHUNK])
    io["out_chunk"] = nc.dram_tensor(
        "out_chunk", [2, 128, NCHUNK], F32, kind="ExternalOutput").ap()
    from contextlib import ExitStack
    with tile.TileContext(nc) as tc, ExitStack() as ctx:
        build_body(ctx, tc, io)
    nc.compile()
    return nc


def prep_weights(i):
    """Host-side preprocessing: transposes, head-channel permutation, bias
    folding (extra contraction rows), K=128 zero padding, BN folding."""
    import ml_dtypes
    bf = ml_dtypes.bfloat16
    f = np.float32
    a = {k: np.asarray(v, dtype=f) for k, v in i.items()}
    # permutation making head channels contiguous: c' = h*64+d  <- c = 4*d+h
    perm = (np.arange(H)[:, None] + H * np.arange(DIM)[None, :]).reshape(-1)

    def w1t(w):       # [D8, D] -> [128, 2, D8]
        return np.ascontiguousarray(w.T.reshape(2, 128, D8).swapaxes(0, 1))

    def w2tp(w, b):   # [D, D8] x [D] -> [128, 2, 128]: rows [w.T; b; zeros]
        o = np.zeros((128, 2, 128), f)
        o[0:D8] = w.T.reshape(D8, 2, 128)
        o[D8] = b.reshape(2, 128)
        return o

    out = {
        "wq1t": w1t(a["Wq1"]), "bq1": a["bq1"].reshape(D8, 1),
        "wk1t": w1t(a["Wk1"]), "bk1": a["bk1"].reshape(D8, 1),
        "wv1t": w1t(a["Wv1"]), "bv1": a["bv1"].reshape(D8, 1),
        "wm2t": w2tp(a["Wm2"], a["bm2"]),
        "wp2t": np.ascontiguousarray(np.concatenate(
            [a["Wp2"].T.reshape(TD8, 2, 128), a["bp2"].reshape(1, 2, 128)], 0)),
        "bm1": a["bm1"].reshape(1, D8),
        "bp1": a["bp1"].reshape(1, TD8),
        "g1s": (a["g1"] / np.sqrt(f(1.0) + f(BN_EPS))).reshape(TD8, 1).astype(f),
        "be1": a["be1"].reshape(TD8, 1),
        "ones": np.ones((1, NCHUNK), f),
    }
    # rvp [128, H*128]: kv1 layout has v1 at rows 64-95, ones at row 96.
    # per head h: cols [128h, 128h+64) = v weights; col 128h+64 = ones col
    # (softmax denominator); cols 128h+65.. zero.
    wv2p, bv2p = a["Wv2"][perm], a["bv2"][perm]
    rvp = np.zeros((128, H * 128), f)
    for h in range(H):
        c0 = h * 128
        rvp[64:64 + D8, c0 + 1:c0 + 1 + DIM] = wv2p[h * DIM:(h + 1) * DIM].T
        rvp[96, c0 + 1:c0 + 1 + DIM] = bv2p[h * DIM:(h + 1) * DIM]
        rvp[96, c0] = 1.0
    out["rvp"] = rvp
    # cht[:, h, :] = C_h^T (zero-padded to 128x128), C_h = A_h @ B_h.T where
    # A/B are the bias-extended per-head blocks of Wk2'/Wq2' transposed.
    wq2e = np.concatenate([a["Wq2"][perm].T, a["bq2"][perm][None, :]], 0)  # [33, 256]
    wk2e = np.concatenate([a["Wk2"][perm].T, a["bk2"][perm][None, :]], 0)
    cht = np.zeros((128, H, 128), f)
    for h in range(H):
        A = wk2e[:, h * DIM:(h + 1) * DIM]        # [33, 64]
        Bq = wq2e[:, h * DIM:(h + 1) * DIM]
        C = (A.astype(np.float64) @ Bq.astype(np.float64).T).astype(f)  # [33,33]
        cht[0:D8 + 1, h, 0:D8 + 1] = C.T
    out["cht"] = cht
    # wm1t [128, 4, D8]: [d, h, :] = Wm1'[:, h*64+d] for d<64, zeros below
    wm1p = a["Wm1"][:, perm]
    wm1t = np.zeros((128, H, D8), f)
    wm1t[1:DIM + 1] = wm1p.T.reshape(H, DIM, D8).swapaxes(0, 1)
    out["wm1t"] = wm1t
    # mlp conv1 split into x-part and msg-part
    out["wp1xt"] = np.ascontiguousarray(
        a["Wp1"][:, 0:D].T.reshape(2, 128, TD8).swapaxes(0, 1))
    out["wp1mt"] = np.ascontiguousarray(
        a["Wp1"][:, D:TD].T.reshape(2, 128, TD8).swapaxes(0, 1))
    bf16_names = {"wk1t", "wv1t", "cht", "rvp", "wm1t", "wm2t", "wp1mt"}
    return {k: np.ascontiguousarray(v.astype(bf) if k in bf16_names else v)
            for k, v in out.items()}


_NC_CACHE = None


def _get_nc():
    global _NC_CACHE
    if _NC_CACHE is None:
        _NC_CACHE = build_program()
    return _NC_CACHE


def make_in_maps(inputs):
    import ml_dtypes
    w = prep_weights(inputs)
    x = np.ascontiguousarray(np.asarray(inputs["x"], np.float32))
    src = np.ascontiguousarray(np.asarray(inputs["source"], np.float32))
    in_maps = []
    for c in range(NCORES):
        b, ns = c // 4, (c % 4) * NCHUNK
        m = dict(w)
        m["x_chunk"] = np.ascontiguousarray(
            x[b].reshape(2, 128, N)[:, :, ns:ns + NCHUNK])
        m["source_b"] = np.ascontiguousarray(src[b].reshape(2, 128, M)).astype(
            ml_dtypes.bfloat16)
        in_maps.append(m)
    return in_maps


def assemble_out(results):
    out = np.empty((B, D, N), np.float32)
    for c in range(NCORES):
        b, ns = c // 4, (c % 4) * NCHUNK
        out[b].reshape(2, 128, N)[:, :, ns:ns + NCHUNK] = (
            results[c]["out_chunk"])
    return out


def kernel(**inputs):
    nc = _get_nc()
    res = bass_utils.run_bass_kernel_spmd(
        nc, make_in_maps(inputs), core_ids=list(range(NCORES)))
    return assemble_out(res.results)
